# revision 9
# baseline (speedup 1.0000x reference)
"""Trainium2 Bass kernel for nn_DecoderWithAttention (Show-Attend-Tell decoder).

Strategy: data-parallel over batch across 8 cores (strided row assignment so
every core gets a mix of long/short sequences). Per core B_local=8 rows. All
matmuls run in bf16 with f32 PSUM accumulation. The recurrence is fully
unrolled and specialized at build time to the actual `lengths` (per-step
active-row counts). The vocab projection is deferred: the h history is kept
in SBUF (transposed) and projected once at the end while fc_w streams from
HBM. Outputs for inactive (b, t) are forced to zero with mask multiplies, so
ragged semantics match the reference exactly.

Self-contained: hardcodes all shapes from the problem spec.
"""

import os
import numpy as np
import ml_dtypes

BF16 = ml_dtypes.bfloat16

B, P, ENC = 64, 196, 2048
A, E, D, V = 512, 512, 512, 20000
MAXLEN = 22
T = MAXLEN - 1            # predictions time dim
NCORES = 8
BL = B // NCORES          # local batch rows per core
BP = BL * P               # stacked (b, p) rows per core = 1568
G4 = 4 * D                # gate width 2048

KC_E = ENC // 128         # 16
KC_D = D // 128           # 4
KC_BP = (BP + 127) // 128  # 13 (last chunk 32 rows)
WIHC_RES = 16             # all e-chunks of w_ih_ctx resident

_prog_cache = {}
LAST_RESULT = None


def _cdiv(a, b):
    return (a + b - 1) // b


def _splits(total, maxn=512):
    out, s = [], 0
    while s < total:
        n = min(maxn, total - s)
        out.append((s, n))
        s += n
    return out


def _build_program(n_ts, t_eff):
    import concourse.bacc as bacc
    import concourse.mybir as mybir
    from concourse.tile import TileContext

    dt = mybir.dt
    AF = mybir.ActivationFunctionType
    OP = mybir.AluOpType

    nc = bacc.Bacc("TRN2", target_bir_lowering=False, debug=False)

    def din(name, shape, d=dt.bfloat16):
        return nc.dram_tensor(name, shape, d, kind="ExternalInput").ap()

    NTJ = 8 * t_eff
    TJCH = _splits(NTJ, 128)          # gates_e row chunks (t-major)
    FC_JCH = [(0, 5), (5, 8)]         # fc row blocks (whole local rows)

    enc_d = din("enc", [BP, ENC])
    encT_d = din("encT", [ENC, BP])
    encw_d = din("encw", [ENC, A])
    wihc_d = din("wihc", [ENC, G4])
    whh_d = din("whh", [D, G4])
    fbeta_d = din("fbeta", [D, ENC])
    fbetab_d = din("fbetab", [128, KC_E], dt.float32)
    dattw_d = din("dattw", [D, A])
    dattb_d = din("dattb", [128, KC_D], dt.float32)
    wrep_d = din("wrep", [A, BL])
    embT_d = din("embT", [E + 1, NTJ])
    w1aug_d = din("w1aug", [E + 1, G4])
    i168_d = din("i168", [NTJ, NTJ])
    eye8_d = din("eye8", [BL, BL])
    h0T_d = din("h0T", [D, BL])
    c0_d = din("c0", [BL, D], dt.float32)
    amask_d = din("amask", [BL, t_eff], dt.float32)
    mfc_d = din("mfc", [BL * t_eff, 1], dt.float32)
    fcw_d = din("fcw", [D + 1, V])

    preds_d = nc.dram_tensor("preds", [BL * T, V], dt.float32,
                             kind="ExternalOutput").ap()
    alphas_d = nc.dram_tensor("alphas", [BL, T, P], dt.float32,
                              kind="ExternalOutput").ap()

    # block-diag alpha fill segments: (j, chunk, r0, r1, p0)
    bd_segs = []
    for j in range(BL):
        r, p0 = 196 * j, 0
        while p0 < P:
            c = r // 128
            take = min(P - p0, 128 - (r - 128 * c))
            bd_segs.append((j, c, r - 128 * c, r - 128 * c + take, p0))
            r += take
            p0 += take

    HSLOT = 8 * (t_eff + 1)

    with TileContext(nc) as tc:
        with tc.sbuf_pool(name="keep", bufs=1) as kp:
            HT_sb = kp.tile([128, KC_D * HSLOT], dt.bfloat16)
            att1_sb = kp.tile([128, 4 * BP], dt.bfloat16)
            ge_sb = kp.tile([128, len(TJCH) * G4], dt.bfloat16)
            i168_sb = kp.tile([128, len(TJCH) * NTJ], dt.bfloat16)
            bd_sb = kp.tile([128, KC_BP * BL], dt.bfloat16)
            c_sb = kp.tile([BL, D], dt.float32)
            dattb_sb = kp.tile([128, KC_D], dt.float32)
            fbetab_sb = kp.tile([128, KC_E], dt.float32)
            wrep_sb = kp.tile([128, KC_D * BL], dt.bfloat16)
            eye8_sb = kp.tile([BL, BL], dt.bfloat16)
            amask_sb = kp.tile([BL, t_eff], dt.float32)
            mfc_sb = kp.tile([128, 2], dt.float32)
            ones1 = kp.tile([1, max(NTJ, 8)], dt.bfloat16)
            sc_sb = kp.tile([BL, P], dt.float32)

            for c in range(KC_D):
                nc.sync.dma_start(out=HT_sb[:, c * HSLOT:c * HSLOT + BL],
                                  in_=h0T_d[128 * c:128 * (c + 1), :])
                nc.sync.dma_start(out=wrep_sb[:, c * BL:(c + 1) * BL],
                                  in_=wrep_d[128 * c:128 * (c + 1), :])
            for ci, (s, n) in enumerate(TJCH):
                nc.sync.dma_start(out=i168_sb[:n, ci * NTJ:ci * NTJ + NTJ],
                                  in_=i168_d[s:s + n, :])
            nc.sync.dma_start(out=dattb_sb[:], in_=dattb_d[:])
            nc.sync.dma_start(out=fbetab_sb[:], in_=fbetab_d[:])
            nc.sync.dma_start(out=eye8_sb[:], in_=eye8_d[:])
            nc.sync.dma_start(out=c_sb[:], in_=c0_d[:])
            nc.sync.dma_start(out=amask_sb[:], in_=amask_d[:])
            for m, (j0, j1) in enumerate(FC_JCH):
                nr = (j1 - j0) * t_eff
                nc.sync.dma_start(out=mfc_sb[:nr, m:m + 1],
                                  in_=mfc_d[j0 * t_eff:j1 * t_eff, :])
            nc.vector.memset(bd_sb[:], 0.0)
            nc.vector.memset(ones1[:], 1.0)
            nc.vector.memset(sc_sb[:], 0.0)

            # ======== phase A: att1 and gates_e ========
            with tc.sbuf_pool(name="pha", bufs=1) as ap, \
                 tc.sbuf_pool(name="phas", bufs=2) as asp, \
                 tc.psum_pool(name="phap", bufs=1) as aq:
                encw_sb = ap.tile([128, KC_E * A], dt.bfloat16)
                w1_sb = ap.tile([128, 4 * G4], dt.bfloat16)
                w1r_sb = ap.tile([1, G4], dt.bfloat16)
                embT_sb = ap.tile([128, 4 * NTJ], dt.bfloat16)
                embTr_sb = ap.tile([1, NTJ], dt.bfloat16)
                for c in range(KC_E):
                    nc.sync.dma_start(out=encw_sb[:, c * A:(c + 1) * A],
                                      in_=encw_d[128 * c:128 * (c + 1), :])
                for c in range(4):
                    nc.sync.dma_start(out=w1_sb[:, c * G4:(c + 1) * G4],
                                      in_=w1aug_d[128 * c:128 * (c + 1), :])
                    nc.sync.dma_start(out=embT_sb[:, c * NTJ:(c + 1) * NTJ],
                                      in_=embT_d[128 * c:128 * (c + 1), :])
                nc.sync.dma_start(out=w1r_sb[:], in_=w1aug_d[E:E + 1, :])
                nc.sync.dma_start(out=embTr_sb[:], in_=embT_d[E:E + 1, :])

                for m in range(4):          # a-chunks of att1
                    ps = aq.tile([128, BP], dt.float32, tag="aps")
                    for k in range(KC_E):
                        rhs = asp.tile([128, BP], dt.bfloat16, tag="encT")
                        nc.sync.dma_start(
                            out=rhs[:], in_=encT_d[128 * k:128 * (k + 1), :])
                        for (s, n) in _splits(BP):
                            nc.tensor.matmul(
                                ps[:, s:s + n],
                                encw_sb[:, k * A + 128 * m:k * A + 128 * (m + 1)],
                                rhs[:, s:s + n],
                                start=(k == 0), stop=(k == KC_E - 1))
                    if m % 2 == 0:
                        nc.scalar.copy(att1_sb[:, m * BP:(m + 1) * BP], ps[:])
                    else:
                        nc.vector.tensor_copy(att1_sb[:, m * BP:(m + 1) * BP],
                                              ps[:])

                for mi, (s, n) in enumerate(TJCH):   # gates_e rows (t-major)
                    ps = aq.tile([128, G4], dt.float32, tag="aps")
                    for (vs, vn) in _splits(G4):
                        for k in range(5):
                            lhs = (embT_sb[:, k * NTJ + s:k * NTJ + s + n]
                                   if k < 4 else embTr_sb[:, s:s + n])
                            rhs = (w1_sb[:, k * G4 + vs:k * G4 + vs + vn]
                                   if k < 4 else w1r_sb[:, vs:vs + vn])
                            nc.tensor.matmul(ps[:n, vs:vs + vn], lhs, rhs,
                                             start=(k == 0), stop=(k == 4))
                    if mi % 2 == 0:
                        nc.scalar.copy(ge_sb[:n, mi * G4:(mi + 1) * G4],
                                       ps[:n, :])
                    else:
                        nc.vector.tensor_copy(ge_sb[:n, mi * G4:(mi + 1) * G4],
                                              ps[:n, :])

            # ======== recurrent loop ========
            with tc.sbuf_pool(name="lw", bufs=1) as lw, \
                 tc.sbuf_pool(name="rp2", bufs=2) as rpp, \
                 tc.sbuf_pool(name="lt", bufs=1) as lt, \
                 tc.psum_pool(name="bigp", bufs=1) as bigp, \
                 tc.psum_pool(name="smallp", bufs=4) as smp:

                enc_sb = lw.tile([128, KC_BP * ENC], dt.bfloat16)
                wihcr_sb = lw.tile([128, WIHC_RES * G4], dt.bfloat16)
                whh_sb = lw.tile([128, KC_D * G4], dt.bfloat16)
                fbeta_sb = lw.tile([128, KC_D * ENC], dt.bfloat16)
                dattw_sb = lw.tile([128, KC_D * A], dt.bfloat16)
                for c in range(KC_BP):
                    rows = min(128, BP - 128 * c)
                    nc.sync.dma_start(out=enc_sb[:rows, c * ENC:(c + 1) * ENC],
                                      in_=enc_d[128 * c:128 * c + rows, :])
                for c in range(WIHC_RES):
                    nc.sync.dma_start(out=wihcr_sb[:, c * G4:(c + 1) * G4],
                                      in_=wihc_d[128 * c:128 * (c + 1), :])
                for c in range(KC_D):
                    nc.sync.dma_start(out=whh_sb[:, c * G4:(c + 1) * G4],
                                      in_=whh_d[128 * c:128 * (c + 1), :])
                    nc.sync.dma_start(out=fbeta_sb[:, c * ENC:(c + 1) * ENC],
                                      in_=fbeta_d[128 * c:128 * (c + 1), :])
                    nc.sync.dma_start(out=dattw_sb[:, c * A:(c + 1) * A],
                                      in_=dattw_d[128 * c:128 * (c + 1), :])

                for t in range(t_eff):
                    nt = n_ts[t]
                    if nt == 0:
                        break
                    hs = [HT_sb[:, c * HSLOT + 8 * t:c * HSLOT + 8 * t + BL]
                          for c in range(KC_D)]

                    # --- att2 = dec_att_w.T @ h + b  -> (a, b) f32 ---
                    att2_sb = lt.tile([128, 4 * BL], dt.float32, tag="att2")
                    for m in range(4):
                        ps = smp.tile([128, BL], dt.float32, tag="sp")
                        for k in range(KC_D):
                            nc.tensor.matmul(
                                ps[:],
                                dattw_sb[:, k * A + 128 * m:k * A + 128 * (m + 1)],
                                hs[k], start=(k == 0), stop=(k == KC_D - 1))
                        nc.scalar.activation(att2_sb[:, m * BL:(m + 1) * BL],
                                             ps[:], AF.Identity,
                                             bias=dattb_sb[:, m:m + 1])

                    # --- relu_pre per a-chunk; scores matmul ---
                    sps = bigp.tile([BL, G4], dt.float32, tag="bigps")
                    ncol = P * nt
                    nspl = _splits(ncol)
                    for m in range(4):
                        rp = rpp.tile([128, BP], dt.bfloat16, tag="rp")
                        for b in range(nt):
                            nc.scalar.activation(
                                rp[:, b * P:(b + 1) * P],
                                att1_sb[:, m * BP + b * P:m * BP + (b + 1) * P],
                                AF.Relu,
                                bias=att2_sb[:, m * BL + b:m * BL + b + 1])
                        for (s, n) in nspl:
                            nc.tensor.matmul(sps[:nt, s:s + n],
                                             wrep_sb[:, m * BL:m * BL + nt],
                                             rp[:, s:s + n],
                                             start=(m == 0), stop=(m == 3))

    # --- extract per-row scores; softmax over p ---
                    scf = lt.tile([BL, G4], dt.float32, tag="scf")
                    half = (ncol // 2) & ~3
                    if half > 0:
                        nc.scalar.copy(scf[:nt, 0:half], sps[:nt, 0:half])
                    nc.vector.tensor_copy(scf[:nt, half:ncol],
                                          sps[:nt, half:ncol])
                    for b in range(nt):
                        nc.sync.dma_start(out=sc_sb[b:b + 1, :],
                                          in_=scf[b:b + 1, b * P:(b + 1) * P])
                    nmax = lt.tile([BL, 1], dt.float32, tag="nmax")
                    nc.vector.tensor_reduce(nmax[:], sc_sb[:],
                                            mybir.AxisListType.X, OP.max,
                                            negate=True)
                    ex = lt.tile([BL, P], dt.float32, tag="ex")
                    esum = lt.tile([BL, 1], dt.float32, tag="esum")
                    nc.scalar.activation(ex[:], sc_sb[:], AF.Exp,
                                         bias=nmax[:], accum_out=esum[:])
                    rec = lt.tile([BL, 1], dt.float32, tag="rec")
                    nc.vector.reciprocal(rec[:], esum[:])
                    al_f = lt.tile([BL, P], dt.float32, tag="alf")
                    nc.vector.tensor_scalar(al_f[:], ex[:], rec[:],
                                            amask_sb[:, t:t + 1],
                                            op0=OP.mult, op1=OP.mult)
                    al_b = lt.tile([BL, P], dt.bfloat16, tag="alb")
                    nc.vector.tensor_copy(al_b[:], al_f[:])
                    nc.sync.dma_start(out=alphas_d[:, t, :], in_=al_f[:])
                    for (j, c, r0, r1, p0) in bd_segs:
                        if j < nt:
                            nc.sync.dma_start(
                                out=bd_sb[r0:r1, c * BL + j:c * BL + j + 1],
                                in_=al_b[j:j + 1, p0:p0 + (r1 - r0)])

                    # --- ctx_T, gate_T per e-chunk -> x_T bf16 ---
                    xT = lt.tile([128, KC_E * BL], dt.bfloat16, tag="xT")
                    kc_bd = _cdiv(P * nt, 128)
                    for m in range(KC_E):
                        cps = smp.tile([128, BL], dt.float32, tag="sp")
                        for k in range(kc_bd):
                            rows = min(128, BP - 128 * k)
                            nc.tensor.matmul(
                                cps[:],
                                enc_sb[:rows,
                                       k * ENC + 128 * m:k * ENC + 128 * (m + 1)],
                                bd_sb[:rows, k * BL:(k + 1) * BL],
                                start=(k == 0), stop=(k == kc_bd - 1))
                        gps = smp.tile([128, BL], dt.float32, tag="sp")
                        for k in range(KC_D):
                            nc.tensor.matmul(
                                gps[:],
                                fbeta_sb[:, k * ENC + 128 * m:
                                         k * ENC + 128 * (m + 1)],
                                hs[k], start=(k == 0), stop=(k == KC_D - 1))
                        gsig = lt.tile([128, BL], dt.float32, tag="gsig")
                        nc.scalar.activation(gsig[:], gps[:], AF.Sigmoid,
                                             bias=fbetab_sb[:, m:m + 1])
                        nc.vector.tensor_mul(xT[:, m * BL:(m + 1) * BL],
                                             cps[:], gsig[:])

                    # --- gates psum (8, 2048): inject ge[t], x@Wc, h@U ---
                    gps8 = bigp.tile([BL, G4], dt.float32, tag="bigps")
                    ci = next(i for i, (s, n) in enumerate(TJCH)
                              if s <= 8 * t and 8 * t + 8 <= s + n)
                    s0, n0 = TJCH[ci]
                    for (vs, vn) in _splits(G4):
                        nc.tensor.matmul(
                            gps8[:, vs:vs + vn],
                            i168_sb[:n0, ci * NTJ + 8 * t:ci * NTJ + 8 * t + BL],
                            ge_sb[:n0, ci * G4 + vs:ci * G4 + vs + vn],
                            start=True, stop=False)
                        for k in range(KC_E):
                            nc.tensor.matmul(
                                gps8[:, vs:vs + vn],
                                xT[:, k * BL:(k + 1) * BL],
                                wihcr_sb[:, k * G4 + vs:k * G4 + vs + vn],
                                start=False, stop=False)
                        for k in range(KC_D):
                            nc.tensor.matmul(
                                gps8[:, vs:vs + vn], hs[k],
                                whh_sb[:, k * G4 + vs:k * G4 + vs + vn],
                                start=False, stop=(k == KC_D - 1))

                    # --- LSTM cell ---
                    si = lt.tile([BL, D], dt.bfloat16, tag="si")
                    sf = lt.tile([BL, D], dt.bfloat16, tag="sf")
                    tg = lt.tile([BL, D], dt.bfloat16, tag="tg")
                    so = lt.tile([BL, D], dt.bfloat16, tag="so")
                    nc.scalar.activation(si[:], gps8[:, 0:D], AF.Sigmoid)
                    nc.scalar.activation(sf[:], gps8[:, D:2 * D], AF.Sigmoid)
                    nc.scalar.activation(tg[:], gps8[:, 2 * D:3 * D], AF.Tanh)
                    nc.scalar.activation(so[:], gps8[:, 3 * D:4 * D], AF.Sigmoid)
                    m1 = lt.tile([BL, D], dt.float32, tag="m1")
                    m2 = lt.tile([BL, D], dt.float32, tag="m2")
                    nc.vector.tensor_mul(m1[:], sf[:], c_sb[:])
                    nc.vector.tensor_mul(m2[:], si[:], tg[:])
                    nc.vector.tensor_add(c_sb[:], m1[:], m2[:])
                    nc.scalar.activation(m1[:], c_sb[:], AF.Tanh)
                    h_b = lt.tile([BL, D], dt.bfloat16, tag="hb")
                    nc.vector.tensor_mul(h_b[:], so[:], m1[:])

                    # --- transpose h -> H_T slot t+1 ---
                    for c in range(KC_D):
                        tps = smp.tile([128, BL], dt.bfloat16, tag="sp")
                        nc.tensor.transpose(tps[:],
                                            h_b[:, 128 * c:128 * (c + 1)],
                                            eye8_sb[:])
                        dst = HT_sb[:, c * HSLOT + 8 * (t + 1):
                                    c * HSLOT + 8 * (t + 1) + BL]
                        if c % 2 == 0:
                            nc.scalar.copy(dst, tps[:])
                        else:
                            nc.vector.tensor_copy(dst, tps[:])

            # ======== fc projection (deferred) ========
            with tc.sbuf_pool(name="fcs", bufs=1) as fp, \
                 tc.sbuf_pool(name="fcw", bufs=3) as fwp, \
                 tc.sbuf_pool(name="fco", bufs=3) as fop, \
                 tc.psum_pool(name="fcp", bufs=4) as fpp:
                Hjm = fp.tile([128, KC_D * NTJ], dt.bfloat16)
                for c in range(KC_D):
                    region = HT_sb[:, c * HSLOT:(c + 1) * HSLOT]
                    region = region.rearrange("p (t j) -> p t j", j=BL)
                    for j in range(BL):
                        src = region[:, 1:, j]
                        nc.sync.dma_start(
                            out=Hjm[:, c * NTJ + j * t_eff:
                                    c * NTJ + (j + 1) * t_eff],
                            in_=src)
                for (vs, vn) in _splits(V):
                    fw = fwp.tile([128, 4 * 512], dt.bfloat16, tag="fw")
                    fwr = fwp.tile([1, 512], dt.bfloat16, tag="fwr")
                    for k in range(KC_D):
                        nc.sync.dma_start(
                            out=fw[:, 512 * k:512 * k + vn],
                            in_=fcw_d[128 * k:128 * (k + 1), vs:vs + vn])
                    nc.sync.dma_start(out=fwr[:, :vn],
                                      in_=fcw_d[D:D + 1, vs:vs + vn])
                    for m, (j0, j1) in enumerate(FC_JCH):
                        nr = (j1 - j0) * t_eff
                        ms = j0 * t_eff
                        ps = fpp.tile([128, 512], dt.float32, tag="fps")
                        for k in range(KC_D):
                            nc.tensor.matmul(
                                ps[:nr, :vn],
                                Hjm[:, k * NTJ + ms:k * NTJ + ms + nr],
                                fw[:, 512 * k:512 * k + vn],
                                start=(k == 0), stop=False)
                        nc.tensor.matmul(ps[:nr, :vn], ones1[:, ms:ms + nr],
                                         fwr[:, :vn], start=False, stop=True)
                        ob = fop.tile([128, 512], dt.float32, tag="ob")
                        if (vs // 512) % 2 == 0:
                            nc.scalar.activation(ob[:nr, :vn], ps[:nr, :vn],
                                                 AF.Identity,
                                                 scale=mfc_sb[:nr, m:m + 1])
                        else:
                            nc.vector.tensor_scalar_mul(ob[:nr, :vn],
                                                        ps[:nr, :vn],
                                                        mfc_sb[:nr, m:m + 1])
                        for j in range(j0, j1):
                            r = (j - j0) * t_eff
                            nc.sync.dma_start(
                                out=preds_d[j * T:j * T + t_eff, vs:vs + vn],
                                in_=ob[r:r + t_eff, :vn])

    nc.compile()
    return nc


def kernel(**inputs):
    from concourse import bass_utils

    encoder_out = np.asarray(inputs["encoder_out"], np.float32)
    captions = np.asarray(inputs["captions"])
    lengths = np.asarray(inputs["lengths"]).astype(np.int64)
    emb = np.asarray(inputs["emb"], np.float32)
    enc_att_w = np.asarray(inputs["enc_att_w"], np.float32)
    enc_att_b = np.asarray(inputs["enc_att_b"], np.float32)
    dec_att_w = np.asarray(inputs["dec_att_w"], np.float32)
    dec_att_b = np.asarray(inputs["dec_att_b"], np.float32)
    full_att_w = np.asarray(inputs["full_att_w"], np.float32)
    # full_att_b only shifts scores uniformly; softmax is shift-invariant.
    w_ih = np.asarray(inputs["w_ih"], np.float32)
    b_ih = np.asarray(inputs["b_ih"], np.float32)
    w_hh = np.asarray(inputs["w_hh"], np.float32)
    b_hh = np.asarray(inputs["b_hh"], np.float32)
    init_h_w = np.asarray(inputs["init_h_w"], np.float32)
    init_h_b = np.asarray(inputs["init_h_b"], np.float32)
    init_c_w = np.asarray(inputs["init_c_w"], np.float32)
    init_c_b = np.asarray(inputs["init_c_b"], np.float32)
    f_beta_w = np.asarray(inputs["f_beta_w"], np.float32)
    f_beta_b = np.asarray(inputs["f_beta_b"], np.float32)
    fc_w = np.asarray(inputs["fc_w"], np.float32)
    fc_b = np.asarray(inputs["fc_b"], np.float32)

    dec_len = lengths - 1
    t_eff = int(min(dec_len.max(), T))
    n_ts = [_cdiv(int((dec_len > t).sum()), NCORES) for t in range(t_eff)]

    env_t = os.environ.get("KERNEL_TSTEPS")
    if env_t:
        t_eff = min(t_eff, int(env_t))
        n_ts = n_ts[:t_eff]

    key = (tuple(n_ts), t_eff)
    if key not in _prog_cache:
        _prog_cache[key] = _build_program(n_ts, t_eff)
    nc = _prog_cache[key]

    # host-side preamble (identical math to the reference preamble)
    mean_enc = encoder_out.mean(axis=1)
    h0 = mean_enc @ init_h_w + init_h_b
    c0 = mean_enc @ init_c_w + init_c_b
    x_emb = emb[captions[:, :t_eff].astype(np.int64)]      # (B, t_eff, E)
    bias_g = (b_ih + b_hh).astype(np.float32)

    w1aug = np.concatenate([w_ih[:E], bias_g[None, :]], axis=0)
    fcw_aug = np.concatenate([fc_w, fc_b[None, :]], axis=0)
    NTJ = 8 * t_eff
    amask_all = (np.arange(t_eff)[None, :] < dec_len[:, None]).astype(np.float32)

    in_maps = []
    row_list = []
    for k in range(NCORES):
        rows = np.arange(BL) * NCORES + k
        enc_k = np.ascontiguousarray(encoder_out[rows].reshape(BP, ENC))
        embT = np.ascontiguousarray(
            x_emb[rows].transpose(2, 1, 0).reshape(E, NTJ))
        embT_aug = np.concatenate([embT, np.ones((1, NTJ), np.float32)], 0)
        mfc = np.zeros((BL * t_eff, 1), np.float32)
        for j in range(BL):
            mfc[j * t_eff:j * t_eff + int(min(dec_len[rows[j]], t_eff))] = 1.0
        im = {
            "enc": enc_k.astype(BF16),
            "encT": np.ascontiguousarray(enc_k.T).astype(BF16),
            "encw": enc_att_w.astype(BF16),
            "wihc": w_ih[E:].astype(BF16),
            "whh": w_hh.astype(BF16),
            "fbeta": f_beta_w.astype(BF16),
            "fbetab": np.ascontiguousarray(
                f_beta_b.reshape(KC_E, 128).T).astype(np.float32),
            "dattw": dec_att_w.astype(BF16),
            "dattb": np.ascontiguousarray(
                dec_att_b.reshape(KC_D, 128).T).astype(np.float32),
            "wrep": np.repeat(full_att_w, BL, axis=1).astype(BF16),
            "embT": embT_aug.astype(BF16),
            "w1aug": w1aug.astype(BF16),
            "i168": np.eye(NTJ, dtype=np.float32).astype(BF16),
            "eye8": np.eye(BL, dtype=np.float32).astype(BF16),
            "h0T": np.ascontiguousarray(h0[rows].T).astype(BF16),
            "c0": np.ascontiguousarray(c0[rows]).astype(np.float32),
            "amask": np.ascontiguousarray(amask_all[rows]),
            "mfc": mfc,
            "fcw": fcw_aug.astype(BF16),
        }
        in_maps.append(im)
        row_list.append(rows)

    res = bass_utils.run_bass_kernel_spmd(nc, in_maps,
                                          core_ids=list(range(NCORES)))
    global LAST_RESULT
    LAST_RESULT = res

    predictions = np.zeros((B, T, V), np.float32)
    alphas = np.zeros((B, T, P), np.float32)
    for k in range(NCORES):
        predictions[row_list[k]] = res.results[k]["preds"].reshape(BL, T, V)
        alphas[row_list[k]] = res.results[k]["alphas"]
    return predictions, alphas


# revision 11
# speedup vs baseline: 1.0908x; 1.0908x over previous
"""Trainium2 Bass kernel for nn_DecoderWithAttention (Show-Attend-Tell decoder).

Strategy: data-parallel over batch across 8 cores (strided row assignment so
every core gets a mix of long/short sequences). Per core B_local=8 rows. All
matmuls run in bf16 with f32 PSUM accumulation. The recurrence is fully
unrolled and specialized at build time to the actual `lengths` (per-step
active-row counts). The vocab projection is deferred: the h history is kept
in SBUF (transposed) and projected once at the end while fc_w streams from
HBM. Outputs for inactive (b, t) are forced to zero with mask multiplies, so
ragged semantics match the reference exactly.

Self-contained: hardcodes all shapes from the problem spec.
"""

import os
import numpy as np
import ml_dtypes

BF16 = ml_dtypes.bfloat16

B, P, ENC = 64, 196, 2048
A, E, D, V = 512, 512, 512, 20000
MAXLEN = 22
T = MAXLEN - 1            # predictions time dim
NCORES = 8
BL = B // NCORES          # local batch rows per core
BP = BL * P               # stacked (b, p) rows per core = 1568
G4 = 4 * D                # gate width 2048

KC_E = ENC // 128         # 16
KC_D = D // 128           # 4
KC_BP = (BP + 127) // 128  # 13 (last chunk 32 rows)
WIHC_RES = 16             # all e-chunks of w_ih_ctx resident

_prog_cache = {}
LAST_RESULT = None


def _cdiv(a, b):
    return (a + b - 1) // b


def _splits(total, maxn=512):
    out, s = [], 0
    while s < total:
        n = min(maxn, total - s)
        out.append((s, n))
        s += n
    return out


def _build_program(n_ts, t_eff):
    import concourse.bacc as bacc
    import concourse.mybir as mybir
    from concourse.tile import TileContext

    dt = mybir.dt
    AF = mybir.ActivationFunctionType
    OP = mybir.AluOpType

    nc = bacc.Bacc("TRN2", target_bir_lowering=False, debug=False)

    def din(name, shape, d=dt.bfloat16):
        return nc.dram_tensor(name, shape, d, kind="ExternalInput").ap()

    NTJ = 8 * t_eff
    TJCH = _splits(NTJ, 128)          # gates_e row chunks (t-major)
    FC_JCH = [(0, 5), (5, 8)]         # fc row blocks (whole local rows)

    enc_d = din("enc", [BP, ENC])
    encT_d = din("encT", [ENC, BP])
    encw_d = din("encw", [ENC, A])
    wihc_d = din("wihc", [ENC, G4])
    whh_d = din("whh", [D, G4])
    fbeta_d = din("fbeta", [D, ENC])
    fbetab_d = din("fbetab", [128, KC_E], dt.float32)
    dattw_d = din("dattw", [D, A])
    dattb_d = din("dattb", [128, KC_D], dt.float32)
    wrep_d = din("wrep", [A, BL])
    embT_d = din("embT", [E + 1, NTJ])
    w1aug_d = din("w1aug", [E + 1, G4])
    i168_d = din("i168", [NTJ, NTJ])
    eye8_d = din("eye8", [BL, BL])
    h0T_d = din("h0T", [D, BL])
    c0_d = din("c0", [BL, D], dt.float32)
    amask_d = din("amask", [BL, t_eff], dt.float32)
    mfc_d = din("mfc", [BL * t_eff, 1], dt.float32)
    fcw_d = din("fcw", [D + 1, V])

    preds_d = nc.dram_tensor("preds", [BL * T, V], dt.float32,
                             kind="ExternalOutput").ap()
    alphas_d = nc.dram_tensor("alphas", [BL, T, P], dt.float32,
                              kind="ExternalOutput").ap()

    # block-diag alpha fill segments: (j, chunk, r0, r1, p0)
    bd_segs = []
    for j in range(BL):
        r, p0 = 196 * j, 0
        while p0 < P:
            c = r // 128
            take = min(P - p0, 128 - (r - 128 * c))
            bd_segs.append((j, c, r - 128 * c, r - 128 * c + take, p0))
            r += take
            p0 += take

    HSLOT = 8 * (t_eff + 1)

    with TileContext(nc) as tc:
        with tc.sbuf_pool(name="keep", bufs=1) as kp:
            HT_sb = kp.tile([128, KC_D * HSLOT], dt.bfloat16)
            att1_sb = kp.tile([128, 4 * BP], dt.bfloat16)
            ge_sb = kp.tile([128, len(TJCH) * G4], dt.bfloat16)
            i168_sb = kp.tile([128, len(TJCH) * NTJ], dt.bfloat16)
            bd_sb = kp.tile([128, KC_BP * BL], dt.bfloat16)
            c_sb = kp.tile([BL, D], dt.float32)
            dattb_sb = kp.tile([128, KC_D], dt.float32)
            fbetab_sb = kp.tile([128, KC_E], dt.float32)
            wrep_sb = kp.tile([128, KC_D * BL], dt.bfloat16)
            eye8_sb = kp.tile([BL, BL], dt.bfloat16)
            amask_sb = kp.tile([BL, t_eff], dt.float32)
            mfc_sb = kp.tile([128, 2], dt.float32)
            ones1 = kp.tile([1, max(NTJ, 8)], dt.bfloat16)
            sc_sb = kp.tile([BL, P], dt.float32)
            zeros196 = kp.tile([128, P], dt.bfloat16)
            gsig_sb = kp.tile([128, KC_E * BL], dt.float32)

            for c in range(KC_D):
                nc.sync.dma_start(out=HT_sb[:, c * HSLOT:c * HSLOT + BL],
                                  in_=h0T_d[128 * c:128 * (c + 1), :])
                nc.sync.dma_start(out=wrep_sb[:, c * BL:(c + 1) * BL],
                                  in_=wrep_d[128 * c:128 * (c + 1), :])
            for ci, (s, n) in enumerate(TJCH):
                nc.sync.dma_start(out=i168_sb[:n, ci * NTJ:ci * NTJ + NTJ],
                                  in_=i168_d[s:s + n, :])
            nc.sync.dma_start(out=dattb_sb[:], in_=dattb_d[:])
            nc.sync.dma_start(out=fbetab_sb[:], in_=fbetab_d[:])
            nc.sync.dma_start(out=eye8_sb[:], in_=eye8_d[:])
            nc.sync.dma_start(out=c_sb[:], in_=c0_d[:])
            nc.sync.dma_start(out=amask_sb[:], in_=amask_d[:])
            for m, (j0, j1) in enumerate(FC_JCH):
                nr = (j1 - j0) * t_eff
                nc.sync.dma_start(out=mfc_sb[:nr, m:m + 1],
                                  in_=mfc_d[j0 * t_eff:j1 * t_eff, :])
            nc.vector.memset(bd_sb[:], 0.0)
            nc.vector.memset(ones1[:], 1.0)
            nc.vector.memset(sc_sb[:], 0.0)
            nc.vector.memset(zeros196[:], 0.0)

            # ======== phase A: att1 and gates_e ========
            with tc.sbuf_pool(name="pha", bufs=1) as ap, \
                 tc.sbuf_pool(name="phas", bufs=2) as asp, \
                 tc.psum_pool(name="phap", bufs=1) as aq:
                encw_sb = ap.tile([128, KC_E * A], dt.bfloat16)
                w1_sb = ap.tile([128, 4 * G4], dt.bfloat16)
                w1r_sb = ap.tile([1, G4], dt.bfloat16)
                embT_sb = ap.tile([128, 4 * NTJ], dt.bfloat16)
                embTr_sb = ap.tile([1, NTJ], dt.bfloat16)
                for c in range(KC_E):
                    nc.sync.dma_start(out=encw_sb[:, c * A:(c + 1) * A],
                                      in_=encw_d[128 * c:128 * (c + 1), :])
                for c in range(4):
                    nc.sync.dma_start(out=w1_sb[:, c * G4:(c + 1) * G4],
                                      in_=w1aug_d[128 * c:128 * (c + 1), :])
                    nc.sync.dma_start(out=embT_sb[:, c * NTJ:(c + 1) * NTJ],
                                      in_=embT_d[128 * c:128 * (c + 1), :])
                nc.sync.dma_start(out=w1r_sb[:], in_=w1aug_d[E:E + 1, :])
                nc.sync.dma_start(out=embTr_sb[:], in_=embT_d[E:E + 1, :])

                for m in range(4):          # a-chunks of att1
                    ps = aq.tile([128, BP], dt.float32, tag="aps")
                    for k in range(KC_E):
                        rhs = asp.tile([128, BP], dt.bfloat16, tag="encT")
                        nc.sync.dma_start(
                            out=rhs[:], in_=encT_d[128 * k:128 * (k + 1), :])
                        for (s, n) in _splits(BP):
                            nc.tensor.matmul(
                                ps[:, s:s + n],
                                encw_sb[:, k * A + 128 * m:k * A + 128 * (m + 1)],
                                rhs[:, s:s + n],
                                start=(k == 0), stop=(k == KC_E - 1))
                    if m % 2 == 0:
                        nc.scalar.copy(att1_sb[:, m * BP:(m + 1) * BP], ps[:])
                    else:
                        nc.vector.tensor_copy(att1_sb[:, m * BP:(m + 1) * BP],
                                              ps[:])

                for mi, (s, n) in enumerate(TJCH):   # gates_e rows (t-major)
                    ps = aq.tile([128, G4], dt.float32, tag="aps")
                    for (vs, vn) in _splits(G4):
                        for k in range(5):
                            lhs = (embT_sb[:, k * NTJ + s:k * NTJ + s + n]
                                   if k < 4 else embTr_sb[:, s:s + n])
                            rhs = (w1_sb[:, k * G4 + vs:k * G4 + vs + vn]
                                   if k < 4 else w1r_sb[:, vs:vs + vn])
                            nc.tensor.matmul(ps[:n, vs:vs + vn], lhs, rhs,
                                             start=(k == 0), stop=(k == 4))
                    if mi % 2 == 0:
                        nc.scalar.copy(ge_sb[:n, mi * G4:(mi + 1) * G4],
                                       ps[:n, :])
                    else:
                        nc.vector.tensor_copy(ge_sb[:n, mi * G4:(mi + 1) * G4],
                                              ps[:n, :])

            # ======== recurrent loop ========
            with tc.sbuf_pool(name="lw", bufs=1) as lw, \
                 tc.sbuf_pool(name="rp2", bufs=2) as rpp, \
                 tc.sbuf_pool(name="lt", bufs=1) as lt, \
                 tc.psum_pool(name="bigp", bufs=1) as bigp, \
                 tc.psum_pool(name="smallp", bufs=4) as smp:

                enc_sb = lw.tile([128, KC_BP * ENC], dt.bfloat16)
                wihcr_sb = lw.tile([128, WIHC_RES * G4], dt.bfloat16)
                whh_sb = lw.tile([128, KC_D * G4], dt.bfloat16)
                fbeta_sb = lw.tile([128, KC_D * ENC], dt.bfloat16)
                dattw_sb = lw.tile([128, KC_D * A], dt.bfloat16)
                for c in range(KC_BP):
                    rows = min(128, BP - 128 * c)
                    nc.sync.dma_start(out=enc_sb[:rows, c * ENC:(c + 1) * ENC],
                                      in_=enc_d[128 * c:128 * c + rows, :])
                for c in range(WIHC_RES):
                    nc.sync.dma_start(out=wihcr_sb[:, c * G4:(c + 1) * G4],
                                      in_=wihc_d[128 * c:128 * (c + 1), :])
                for c in range(KC_D):
                    nc.sync.dma_start(out=whh_sb[:, c * G4:(c + 1) * G4],
                                      in_=whh_d[128 * c:128 * (c + 1), :])
                    nc.sync.dma_start(out=fbeta_sb[:, c * ENC:(c + 1) * ENC],
                                      in_=fbeta_d[128 * c:128 * (c + 1), :])
                    nc.sync.dma_start(out=dattw_sb[:, c * A:(c + 1) * A],
                                      in_=dattw_d[128 * c:128 * (c + 1), :])

                for t in range(t_eff):
                    nt = n_ts[t]
                    if nt == 0:
                        break
                    hs = [HT_sb[:, c * HSLOT + 8 * t:c * HSLOT + 8 * t + BL]
                          for c in range(KC_D)]

                    # --- att2 = dec_att_w.T @ h + b  -> (a, b) f32 ---
                    att2_sb = lt.tile([128, 4 * BL], dt.float32, tag="att2")
                    for m in range(4):
                        ps = smp.tile([128, BL], dt.float32, tag="sp")
                        for k in range(KC_D):
                            nc.tensor.matmul(
                                ps[:],
                                dattw_sb[:, k * A + 128 * m:k * A + 128 * (m + 1)],
                                hs[k], start=(k == 0), stop=(k == KC_D - 1))
                        nc.scalar.activation(att2_sb[:, m * BL:(m + 1) * BL],
                                             ps[:], AF.Identity,
                                             bias=dattb_sb[:, m:m + 1])

                    # --- relu_pre per a-chunk; scores matmul ---
                    sps = bigp.tile([BL, G4], dt.float32, tag="bigps")
                    ncol = P * nt
                    nspl = _splits(ncol)
                    for m in range(4):
                        rp = rpp.tile([128, BP], dt.bfloat16, tag="rp")
                        for b in range(nt):
                            dst = rp[:, b * P:(b + 1) * P]
                            src = att1_sb[:, m * BP + b * P:m * BP + (b + 1) * P]
                            bias = att2_sb[:, m * BL + b:m * BL + b + 1]
                            if (m * BL + b) % 8 < 4:
                                nc.scalar.activation(dst, src, AF.Relu,
                                                     bias=bias)
                            else:
                                nc.vector.scalar_tensor_tensor(
                                    dst, src, bias, zeros196[:, :P],
                                    op0=OP.add, op1=OP.max)
                        for (si, n) in nspl:
                            nc.tensor.matmul(sps[:nt, si:si + n],
                                             wrep_sb[:, m * BL:m * BL + nt],
                                             rp[:, si:si + n],
                                             start=(m == 0), stop=(m == 3))

                    # --- extract per-row scores; softmax over p ---
                    scf = lt.tile([BL, G4], dt.float32, tag="scf")
                    half = (ncol // 2) & ~3
                    if half > 0:
                        nc.scalar.copy(scf[:nt, 0:half], sps[:nt, 0:half])
                    nc.vector.tensor_copy(scf[:nt, half:ncol],
                                          sps[:nt, half:ncol])
                    for b in range(nt):
                        nc.sync.dma_start(out=sc_sb[b:b + 1, :],
                                            in_=scf[b:b + 1, b * P:(b + 1) * P])
                    nmax = lt.tile([BL, 1], dt.float32, tag="nmax")
                    nc.vector.tensor_reduce(nmax[:], sc_sb[:],
                                            mybir.AxisListType.X, OP.max,
                                            negate=True)
                    ex = lt.tile([BL, P], dt.float32, tag="ex")
                    esum = lt.tile([BL, 1], dt.float32, tag="esum")
                    nc.scalar.activation(ex[:], sc_sb[:], AF.Exp,
                                         bias=nmax[:], accum_out=esum[:])
                    rec = lt.tile([BL, 1], dt.float32, tag="rec")
                    nc.vector.reciprocal(rec[:], esum[:])
                    al_f = lt.tile([BL, P], dt.float32, tag="alf")
                    nc.vector.tensor_scalar(al_f[:], ex[:], rec[:],
                                            amask_sb[:, t:t + 1],
                                            op0=OP.mult, op1=OP.mult)
                    al_b = lt.tile([BL, P], dt.bfloat16, tag="alb")
                    nc.vector.tensor_copy(al_b[:], al_f[:])
                    nc.sync.dma_start(out=alphas_d[:, t, :], in_=al_f[:])
                    for (j, c, r0, r1, p0) in bd_segs:
                        if j < nt:
                            nc.sync.dma_start(
                                out=bd_sb[r0:r1, c * BL + j:c * BL + j + 1],
                                in_=al_b[j:j + 1, p0:p0 + (r1 - r0)])

                    # --- gate_T (h only) + inject + w_hh: cover softmax ---
                    for m in range(KC_E):
                        gps = smp.tile([128, BL], dt.float32, tag="sp")
                        for k in range(KC_D):
                            nc.tensor.matmul(
                                gps[:],
                                fbeta_sb[:, k * ENC + 128 * m:
                                         k * ENC + 128 * (m + 1)],
                                hs[k], start=(k == 0), stop=(k == KC_D - 1))
                        nc.scalar.activation(gsig_sb[:, m * BL:(m + 1) * BL],
                                             gps[:], AF.Sigmoid,
                                             bias=fbetab_sb[:, m:m + 1])

                    gps8 = bigp.tile([BL, G4], dt.float32, tag="bigps")
                    ci = next(i for i, (si, n) in enumerate(TJCH)
                              if si <= 8 * t and 8 * t + 8 <= si + n)
                    s0, n0 = TJCH[ci]
                    for (vs, vn) in _splits(G4):
                        nc.tensor.matmul(
                            gps8[:, vs:vs + vn],
                            i168_sb[:n0, ci * NTJ + 8 * t:ci * NTJ + 8 * t + BL],
                            ge_sb[:n0, ci * G4 + vs:ci * G4 + vs + vn],
                            start=True, stop=False)
                        for k in range(KC_D):
                            nc.tensor.matmul(
                                gps8[:, vs:vs + vn], hs[k],
                                whh_sb[:, k * G4 + vs:k * G4 + vs + vn],
                                start=False, stop=False)

                    # --- ctx_T per e-chunk (needs bd); x_T = gsig*ctx ---
                    xT = lt.tile([128, KC_E * BL], dt.bfloat16, tag="xT")
                    kc_bd = _cdiv(P * nt, 128)
                    for m in range(KC_E):
                        cps = smp.tile([128, BL], dt.float32, tag="sp")
                        for k in range(kc_bd):
                            rows = min(128, BP - 128 * k)
                            nc.tensor.matmul(
                                cps[:],
                                enc_sb[:rows,
                                       k * ENC + 128 * m:k * ENC + 128 * (m + 1)],
                                bd_sb[:rows, k * BL:(k + 1) * BL],
                                start=(k == 0), stop=(k == kc_bd - 1))
                        nc.vector.tensor_mul(xT[:, m * BL:(m + 1) * BL],
                                             cps[:],
                                             gsig_sb[:, m * BL:(m + 1) * BL])

                    # --- x @ w_ih_ctx into gates ---
                    for (vs, vn) in _splits(G4):
                        for k in range(KC_E):
                            nc.tensor.matmul(
                                gps8[:, vs:vs + vn],
                                xT[:, k * BL:(k + 1) * BL],
                                wihcr_sb[:, k * G4 + vs:k * G4 + vs + vn],
                                start=False,
                                stop=(k == KC_E - 1))

                    # --- LSTM cell ---
                    si = lt.tile([BL, D], dt.bfloat16, tag="si")
                    sf = lt.tile([BL, D], dt.bfloat16, tag="sf")
                    tg = lt.tile([BL, D], dt.bfloat16, tag="tg")
                    so = lt.tile([BL, D], dt.bfloat16, tag="so")
                    nc.scalar.activation(si[:], gps8[:, 0:D], AF.Sigmoid)
                    nc.scalar.activation(sf[:], gps8[:, D:2 * D], AF.Sigmoid)
                    nc.scalar.activation(tg[:], gps8[:, 2 * D:3 * D], AF.Tanh)
                    nc.scalar.activation(so[:], gps8[:, 3 * D:4 * D], AF.Sigmoid)
                    m1 = lt.tile([BL, D], dt.float32, tag="m1")
                    m2 = lt.tile([BL, D], dt.float32, tag="m2")
                    nc.vector.tensor_mul(m1[:], sf[:], c_sb[:])
                    nc.vector.tensor_mul(m2[:], si[:], tg[:])
                    nc.vector.tensor_add(c_sb[:], m1[:], m2[:])
                    nc.scalar.activation(m1[:], c_sb[:], AF.Tanh)
                    h_b = lt.tile([BL, D], dt.bfloat16, tag="hb")
                    nc.vector.tensor_mul(h_b[:], so[:], m1[:])

                    # --- transpose h -> H_T slot t+1 ---
                    for c in range(KC_D):
                        tps = smp.tile([128, BL], dt.bfloat16, tag="sp")
                        nc.tensor.transpose(tps[:],
                                            h_b[:, 128 * c:128 * (c + 1)],
                                            eye8_sb[:])
                        dst = HT_sb[:, c * HSLOT + 8 * (t + 1):
                                    c * HSLOT + 8 * (t + 1) + BL]
                        if c % 2 == 0:
                            nc.scalar.copy(dst, tps[:])
                        else:
                            nc.vector.tensor_copy(dst, tps[:])

            # ======== fc projection (deferred) ========
            with tc.sbuf_pool(name="fcs", bufs=1) as fp, \
                 tc.sbuf_pool(name="fcw", bufs=3) as fwp, \
                 tc.sbuf_pool(name="fco", bufs=3) as fop, \
                 tc.psum_pool(name="fcp", bufs=4) as fpp:
                Hjm = fp.tile([128, KC_D * NTJ], dt.bfloat16)
                for c in range(KC_D):
                    region = HT_sb[:, c * HSLOT:(c + 1) * HSLOT]
                    region = region.rearrange("p (t j) -> p t j", j=BL)
                    for j in range(BL):
                        src = region[:, 1:, j]
                        nc.sync.dma_start(
                            out=Hjm[:, c * NTJ + j * t_eff:
                                    c * NTJ + (j + 1) * t_eff],
                            in_=src)
                for (vs, vn) in _splits(V):
                    fw = fwp.tile([128, 4 * 512], dt.bfloat16, tag="fw")
                    fwr = fwp.tile([1, 512], dt.bfloat16, tag="fwr")
                    for k in range(KC_D):
                        nc.sync.dma_start(
                            out=fw[:, 512 * k:512 * k + vn],
                            in_=fcw_d[128 * k:128 * (k + 1), vs:vs + vn])
                    nc.sync.dma_start(out=fwr[:, :vn],
                                      in_=fcw_d[D:D + 1, vs:vs + vn])
                    for m, (j0, j1) in enumerate(FC_JCH):
                        nr = (j1 - j0) * t_eff
                        ms = j0 * t_eff
                        ps = fpp.tile([128, 512], dt.float32, tag="fps")
                        for k in range(KC_D):
                            nc.tensor.matmul(
                                ps[:nr, :vn],
                                Hjm[:, k * NTJ + ms:k * NTJ + ms + nr],
                                fw[:, 512 * k:512 * k + vn],
                                start=(k == 0), stop=False)
                        nc.tensor.matmul(ps[:nr, :vn], ones1[:, ms:ms + nr],
                                         fwr[:, :vn], start=False, stop=True)
                        ob = fop.tile([128, 512], dt.float32, tag="ob")
                        if (vs // 512) % 2 == 0:
                            nc.scalar.activation(ob[:nr, :vn], ps[:nr, :vn],
                                                 AF.Identity,
                                                 scale=mfc_sb[:nr, m:m + 1])
                        else:
                            nc.vector.tensor_scalar_mul(ob[:nr, :vn],
                                                        ps[:nr, :vn],
                                                        mfc_sb[:nr, m:m + 1])
                        for j in range(j0, j1):
                            r = (j - j0) * t_eff
                            nc.sync.dma_start(
                                out=preds_d[j * T:j * T + t_eff, vs:vs + vn],
                                in_=ob[r:r + t_eff, :vn])

    nc.compile()
    return nc


def kernel(**inputs):
    from concourse import bass_utils

    encoder_out = np.asarray(inputs["encoder_out"], np.float32)
    captions = np.asarray(inputs["captions"])
    lengths = np.asarray(inputs["lengths"]).astype(np.int64)
    emb = np.asarray(inputs["emb"], np.float32)
    enc_att_w = np.asarray(inputs["enc_att_w"], np.float32)
    enc_att_b = np.asarray(inputs["enc_att_b"], np.float32)
    dec_att_w = np.asarray(inputs["dec_att_w"], np.float32)
    dec_att_b = np.asarray(inputs["dec_att_b"], np.float32)
    full_att_w = np.asarray(inputs["full_att_w"], np.float32)
    # full_att_b only shifts scores uniformly; softmax is shift-invariant.
    w_ih = np.asarray(inputs["w_ih"], np.float32)
    b_ih = np.asarray(inputs["b_ih"], np.float32)
    w_hh = np.asarray(inputs["w_hh"], np.float32)
    b_hh = np.asarray(inputs["b_hh"], np.float32)
    init_h_w = np.asarray(inputs["init_h_w"], np.float32)
    init_h_b = np.asarray(inputs["init_h_b"], np.float32)
    init_c_w = np.asarray(inputs["init_c_w"], np.float32)
    init_c_b = np.asarray(inputs["init_c_b"], np.float32)
    f_beta_w = np.asarray(inputs["f_beta_w"], np.float32)
    f_beta_b = np.asarray(inputs["f_beta_b"], np.float32)
    fc_w = np.asarray(inputs["fc_w"], np.float32)
    fc_b = np.asarray(inputs["fc_b"], np.float32)

    dec_len = lengths - 1
    t_eff = int(min(dec_len.max(), T))
    n_ts = [_cdiv(int((dec_len > t).sum()), NCORES) for t in range(t_eff)]

    env_t = os.environ.get("KERNEL_TSTEPS")
    if env_t:
        t_eff = min(t_eff, int(env_t))
        n_ts = n_ts[:t_eff]

    key = (tuple(n_ts), t_eff)
    if key not in _prog_cache:
        _prog_cache[key] = _build_program(n_ts, t_eff)
    nc = _prog_cache[key]

    # host-side preamble (identical math to the reference preamble)
    mean_enc = encoder_out.mean(axis=1)
    h0 = mean_enc @ init_h_w + init_h_b
    c0 = mean_enc @ init_c_w + init_c_b
    x_emb = emb[captions[:, :t_eff].astype(np.int64)]      # (B, t_eff, E)
    bias_g = (b_ih + b_hh).astype(np.float32)

    w1aug = np.concatenate([w_ih[:E], bias_g[None, :]], axis=0)
    fcw_aug = np.concatenate([fc_w, fc_b[None, :]], axis=0)
    NTJ = 8 * t_eff
    amask_all = (np.arange(t_eff)[None, :] < dec_len[:, None]).astype(np.float32)

    in_maps = []
    row_list = []
    for k in range(NCORES):
        rows = np.arange(BL) * NCORES + k
        enc_k = np.ascontiguousarray(encoder_out[rows].reshape(BP, ENC))
        embT = np.ascontiguousarray(
            x_emb[rows].transpose(2, 1, 0).reshape(E, NTJ))
        embT_aug = np.concatenate([embT, np.ones((1, NTJ), np.float32)], 0)
        mfc = np.zeros((BL * t_eff, 1), np.float32)
        for j in range(BL):
            mfc[j * t_eff:j * t_eff + int(min(dec_len[rows[j]], t_eff))] = 1.0
        im = {
            "enc": enc_k.astype(BF16),
            "encT": np.ascontiguousarray(enc_k.T).astype(BF16),
            "encw": enc_att_w.astype(BF16),
            "wihc": w_ih[E:].astype(BF16),
            "whh": w_hh.astype(BF16),
            "fbeta": f_beta_w.astype(BF16),
            "fbetab": np.ascontiguousarray(
                f_beta_b.reshape(KC_E, 128).T).astype(np.float32),
            "dattw": dec_att_w.astype(BF16),
            "dattb": np.ascontiguousarray(
                dec_att_b.reshape(KC_D, 128).T).astype(np.float32),
            "wrep": np.repeat(full_att_w, BL, axis=1).astype(BF16),
            "embT": embT_aug.astype(BF16),
            "w1aug": w1aug.astype(BF16),
            "i168": np.eye(NTJ, dtype=np.float32).astype(BF16),
            "eye8": np.eye(BL, dtype=np.float32).astype(BF16),
            "h0T": np.ascontiguousarray(h0[rows].T).astype(BF16),
            "c0": np.ascontiguousarray(c0[rows]).astype(np.float32),
            "amask": np.ascontiguousarray(amask_all[rows]),
            "mfc": mfc,
            "fcw": fcw_aug.astype(BF16),
        }
        in_maps.append(im)
        row_list.append(rows)

    res = bass_utils.run_bass_kernel_spmd(nc, in_maps,
                                          core_ids=list(range(NCORES)))
    global LAST_RESULT
    LAST_RESULT = res

    predictions = np.zeros((B, T, V), np.float32)
    alphas = np.zeros((B, T, P), np.float32)
    for k in range(NCORES):
        predictions[row_list[k]] = res.results[k]["preds"].reshape(BL, T, V)
        alphas[row_list[k]] = res.results[k]["alphas"]
    return predictions, alphas
